# revision 16
# baseline (speedup 1.0000x reference)
"""CBAM (channel + spatial attention) Trainium2 kernel, 8-core data parallel.

Problem: f [8, 8, 256, 56, 56] f32 -> out same shape.
  x = f.reshape(BT, C, H, W)
  ca = sigmoid(mlp(max_hw(x)) + mlp(mean_hw(x)));  xc = ca * x
  s  = conv7x7([mean_c(xc); max_c(xc)]);           out = sigmoid(s) * xc

Strategy (per NeuronCore, 8 frames each, no collectives):
  - channel-major layout [128 part(channels), free(hw=3136)], 2 channel tiles
  - phase1 (ACT): xb = bf16(x/HW) fused with accum_out -> channel avg
  - pool-max (DVE): log2 fold tree on xb (bf16 TT max at 2x rate)
  - MLP on PE (K-split matmuls), relu rescale trick absorbs the 1/HW scale
  - xc = ca*xb on DVE (bf16 tensor_scalar 4x)
  - spatial sum: PE matmul with ca as lhsT over xb (reduces channel/K dim)
  - spatial max: DVE channel fold + PE 128x112 transposes + DVE reduce from PSUM
  - conv 7x7 as 7 accumulating PE matmuls with host-built banded lhsT [124,56]
  - sa broadcast via PE ones(=HW)-matmul; final mul on DVE (bf16 TT 2x)
  - store via SWDGE dma cast bf16->f32
"""

import os
import sys
from contextlib import ExitStack

import numpy as np

if "/opt/trn_rl_repo" not in sys.path:
    sys.path.insert(0, "/opt/trn_rl_repo")

import concourse.bass as bass
import concourse.tile as tile
from concourse import bacc, mybir
from concourse.bass_utils import run_bass_kernel_spmd
from concourse.masks import make_identity

F32 = mybir.dt.float32
BF16 = mybir.dt.bfloat16

N_CORES = 8
B, T, C, H, W = 8, 8, 256, 56, 56
HW = H * W            # 3136
FRAMES = B * T        # 64
FPC = FRAMES // N_CORES  # frames per core = 8
PAD = 3
HP, WP = H + 2 * PAD, W + 2 * PAD  # 62, 62
CHK = 112             # hw chunk size for transposes (28 * 112 = 3136)
NCHUNK = HW // CHK    # 28
TGRP = 7              # transpose chunks per psum group
NGRP = NCHUNK // TGRP  # 4
SCHK = 448            # free-dim chunk for ssum / broadcast matmuls
NSCHK = HW // SCHK    # 7


def _build_conv_lhsT(conv_w: np.ndarray) -> np.ndarray:
    """Banded matrices for the 7x7 conv as 7 accumulating matmuls over y.

    B[dx] : [124, 56], rows = c*62 + y_in, cols = y_out.
    B[dx][c*62 + yi, yo] = w_eff[c, yi-yo, dx] for 0 <= yi-yo <= 6.
    The channel-mean 1/C is folded into the avg branch (c=0).
    """
    w_eff = conv_w[0].astype(np.float64).copy()  # [2, 7, 7]
    w_eff[0] /= C
    Bm = np.zeros((7, 2 * HP, H), dtype=np.float32)
    dyi = np.arange(7)
    for dx in range(7):
        for c in range(2):
            for yo in range(H):
                Bm[dx, c * HP + yo + dyi, yo] = w_eff[c, :, dx]
    return Bm


def build_nc(n_frames: int = FPC):
    nc = bacc.Bacc("TRN2", target_bir_lowering=False, debug=False,
                   num_devices=N_CORES)

    x_ext = nc.dram_tensor("x", [n_frames, C, HW], F32, kind="ExternalInput")
    w1_ext = nc.dram_tensor("w1", [C, 16], F32, kind="ExternalInput")
    w2_ext = nc.dram_tensor("w2", [16, C], F32, kind="ExternalInput")
    cb_ext = nc.dram_tensor("convb", [7, 2 * HP, H], F32, kind="ExternalInput")
    out_ext = nc.dram_tensor("out", [n_frames, C, HW], F32, kind="ExternalOutput")

    with tile.TileContext(nc) as tc, ExitStack() as ctx:
        consts = ctx.enter_context(tc.tile_pool(name="consts", bufs=1))
        xin = ctx.enter_context(tc.tile_pool(name="xin", bufs=2))
        xbp = ctx.enter_context(tc.tile_pool(name="xb", bufs=2))
        xcp = ctx.enter_context(tc.tile_pool(name="xc", bufs=2))
        obp = ctx.enter_context(tc.tile_pool(name="ob", bufs=2))
        fold = ctx.enter_context(tc.tile_pool(name="fold", bufs=1))
        m1p = ctx.enter_context(tc.tile_pool(name="m1", bufs=2))
        small = ctx.enter_context(tc.tile_pool(name="small", bufs=2))
        sap = ctx.enter_context(tc.tile_pool(name="sa", bufs=1))
        sabp = ctx.enter_context(tc.tile_pool(name="sab", bufs=2))
        # PSUM pools — total across all tags must stay within 8 banks.
        ptr = ctx.enter_context(tc.tile_pool(name="ptr", bufs=2, space="PSUM"))
        pz = ctx.enter_context(tc.tile_pool(name="pz", bufs=1, space="PSUM"))
        pss = ctx.enter_context(tc.tile_pool(name="pss", bufs=1, space="PSUM"))
        pbc = ctx.enter_context(tc.tile_pool(name="pbc", bufs=1, space="PSUM"))
        pmlp = ctx.enter_context(tc.tile_pool(name="pmlp", bufs=1, space="PSUM"))
        # bank budget: ptr 2x2 + pz 1 (shared tag) + pss 1 + pbc 1 + pmlp 1 = 8

        # ---- constants / weights (loaded once) ----
        w1_sb = consts.tile([128, 2, 16], F32)       # [k, ktile, m]
        for t in range(2):
            nc.sync.dma_start(w1_sb[:, t, :], w1_ext[t * 128:(t + 1) * 128, :])
        w2_sb = consts.tile([16, C], F32)
        nc.sync.dma_start(w2_sb[:], w2_ext[:, :])
        cb_sb = consts.tile([124, 7, H], BF16)       # [y_in(+c), dx, y_out]
        nc.gpsimd.dma_start(                          # SWDGE: casts f32->bf16
            cb_sb[:],
            cb_ext.rearrange("d p y -> p d y"),
        )
        ident_b = consts.tile([128, 128], BF16)
        make_identity(nc, ident_b[:])
        ident_f = consts.tile([128, 128], F32)
        make_identity(nc, ident_f[:])
        ones_hw = consts.tile([1, 128], BF16)        # value HW for sa broadcast
        nc.vector.memset(ones_hw[:], float(HW))

        for f in range(n_frames):
            # ---------- load frame ----------
            x_sb = xin.tile([128, 2, HW], F32, tag="x")
            for t in range(2):
                nc.sync.dma_start(
                    x_sb[:, t, :], x_ext[f, t * 128:(t + 1) * 128, :])

            # ---------- phase 1: cast+scale + channel-avg (ACT) ----------
            xb = xbp.tile([128, 2, HW], BF16, tag="xb")
            # pr: flat [128, 4]; col 2t = max of tile t, col 2t+1 = avg of tile t
            pr = small.tile([128, 4], F32, tag="pr")
            for t in range(2):
                nc.scalar.activation(
                    xb[:, t, :], x_sb[:, t, :],
                    mybir.ActivationFunctionType.Copy,
                    scale=1.0 / HW,
                    accum_out=pr[:, 2 * t + 1:2 * t + 2])

            # ---------- pool-max: fold tree on xb (DVE) ----------
            cur = xb
            width = HW
            while width > 98:
                half = width // 2
                nxt = fold.tile([128, 2, half], BF16, tag=f"fold{half}")
                nc.vector.tensor_tensor(
                    out=nxt[:, :, :],
                    in0=cur[:, :, 0:half],
                    in1=cur[:, :, half:2 * half],
                    op=mybir.AluOpType.max)
                cur = nxt
                width = half
            nc.vector.tensor_reduce(
                out=pr.rearrange("p (t two) -> p t two", two=2)[:, :, 0],
                in_=cur[:, :, :],
                axis=mybir.AxisListType.X,
                op=mybir.AluOpType.max)

            # ---------- MLP on PE ----------
            ph = pmlp.tile([16, 2], F32, tag="pmlp")
            for t in range(2):
                nc.tensor.matmul(ph[:], w1_sb[:, t, :],
                                 pr[:, 2 * t:2 * t + 2],
                                 start=(t == 0), stop=(t == 1))
            h = small.tile([16, 2], F32, tag="h")
            # col 0 (max branch) needs the 1/HW scale undone before relu
            nc.scalar.activation(h[:, 0:1], ph[:, 0:1],
                                 mybir.ActivationFunctionType.Relu,
                                 scale=float(HW))
            nc.scalar.activation(h[:, 1:2], ph[:, 1:2],
                                 mybir.ActivationFunctionType.Relu)
            hs = small.tile([16, 1], F32, tag="hs")
            nc.vector.tensor_tensor(out=hs[:], in0=h[:, 0:1], in1=h[:, 1:2],
                                    op=mybir.AluOpType.add)
            pca = pmlp.tile([128, 2], F32, tag="pmlp")
            for t in range(2):
                nc.tensor.matmul(pca[:, t:t + 1],
                                 w2_sb[:, t * 128:(t + 1) * 128], hs[:],
                                 start=True, stop=True)
            ca = small.tile([128, 2], F32, tag="ca")
            nc.scalar.activation(ca[:], pca[:],
                                 mybir.ActivationFunctionType.Sigmoid)
            ca_b = small.tile([128, 2], BF16, tag="ca_b")
            nc.vector.tensor_copy(ca_b[:], ca[:])

            # ---------- xc = ca * xb (DVE, bf16 4x) ----------
            xc = xcp.tile([128, 2, HW], BF16, tag="xc")
            for t in range(2):
                nc.vector.tensor_scalar(
                    out=xc[:, t, :], in0=xb[:, t, :],
                    scalar1=ca[:, t:t + 1], scalar2=None,
                    op0=mybir.AluOpType.mult)

            # ---------- spatial sum via PE (ca as lhsT over xb) ----------
            ssum_row = sap.tile([1, HW], BF16, tag="ssum_row")
            for j in range(NSCHK):
                psc = pss.tile([1, SCHK], F32, tag="pss")
                for t in range(2):
                    nc.tensor.matmul(
                        psc[:],
                        ca_b[:, t:t + 1],
                        xb[:, t, j * SCHK:(j + 1) * SCHK],
                        start=(t == 0), stop=(t == 1))
                nc.scalar.activation(ssum_row[:, j * SCHK:(j + 1) * SCHK],
                                     psc[:],
                                     mybir.ActivationFunctionType.Copy)

            # ---------- spatial max: channel fold + transposes ----------
            m1 = m1p.tile([128, HW], BF16, tag="m1")
            nc.vector.tensor_tensor(out=m1[:], in0=xc[:, 0, :],
                                    in1=xc[:, 1, :], op=mybir.AluOpType.max)
            sm_cols = small.tile([CHK, NCHUNK], F32, tag="sm_cols")
            for g in range(NGRP):
                pt = ptr.tile([CHK, TGRP, 128], BF16, tag="ptr")
                for j in range(TGRP):
                    c = g * TGRP + j
                    nc.tensor.transpose(
                        pt[:, j, :], m1[:, c * CHK:(c + 1) * CHK], ident_b[:])
                nc.vector.tensor_reduce(
                    out=sm_cols[:, g * TGRP:(g + 1) * TGRP],
                    in_=pt[:, :, :],
                    axis=mybir.AxisListType.X,
                    op=mybir.AluOpType.max)

            # sm_cols [112, 28] -> transpose -> [28, 112] -> sbuf (hw order)
            psm = pz.tile([NCHUNK, CHK], F32, tag="pz")
            nc.tensor.transpose(psm[:], sm_cols[:], ident_f[0:CHK, 0:CHK])
            sm_sb = small.tile([NCHUNK, CHK], BF16, tag="sm_sb")
            nc.scalar.activation(sm_sb[:], psm[:],
                                 mybir.ActivationFunctionType.Copy)

            # ---------- conv input assembly ----------
            s_pad = sap.tile([124, WP], BF16, tag="s_pad")
            nc.vector.memset(s_pad[:], 0.0)
            # avg rows (c=0): partitions 3..58; max rows (c=1): 65..120
            # element-count-matched SBUF->SBUF fold DMAs (hw-sequential streams)
            nc.sync.dma_start(s_pad[PAD:PAD + H, PAD:PAD + W], ssum_row[:])
            nc.sync.dma_start(s_pad[HP + PAD:HP + PAD + H, PAD:PAD + W],
                              sm_sb[:])

            # ---------- conv: 7 accumulating matmuls ----------
            pzt = pz.tile([H, W], F32, tag="pz")
            for dx in range(7):
                nc.tensor.matmul(pzt[:], cb_sb[:, dx, :],
                                 s_pad[:, dx:dx + W],
                                 start=(dx == 0), stop=(dx == 6))
            sa_yx = small.tile([H, W], BF16, tag="sa_yx")
            nc.scalar.activation(sa_yx[:], pzt[:],
                                 mybir.ActivationFunctionType.Sigmoid,
                                 scale=float(HW))

            # ---------- sa broadcast ----------
            sa_row = sap.tile([1, HW], BF16, tag="sa_row")
            nc.sync.dma_start(sa_row[:], sa_yx[:])
            sab = sabp.tile([128, HW], BF16, tag="sab")
            for j in range(NSCHK):
                pb = pbc.tile([128, SCHK], F32, tag="pbc")
                nc.tensor.matmul(pb[:], ones_hw[:],
                                 sa_row[:, j * SCHK:(j + 1) * SCHK],
                                 start=True, stop=True)
                nc.scalar.activation(sab[:, j * SCHK:(j + 1) * SCHK], pb[:],
                                     mybir.ActivationFunctionType.Copy)

            # ---------- final: out = xc * (HW * sa) ----------
            ob = obp.tile([128, 2, HW], BF16, tag="ob")
            for t in range(2):
                nc.vector.tensor_tensor(out=ob[:, t, :], in0=xc[:, t, :],
                                        in1=sab[:], op=mybir.AluOpType.mult)
                nc.gpsimd.dma_start(
                    out_ext[f, t * 128:(t + 1) * 128, :], ob[:, t, :])

    nc.finalize()  # bacc register allocation + DCE (bass2jax expects this)
    return nc


_NC_CACHE = {}


def _get_nc(n_frames: int):
    if n_frames not in _NC_CACHE:
        _NC_CACHE[n_frames] = build_nc(n_frames)
    return _NC_CACHE[n_frames]


def kernel(f: np.ndarray, w1: np.ndarray, w2: np.ndarray,
           conv_w: np.ndarray) -> np.ndarray:
    f = np.ascontiguousarray(np.asarray(f, dtype=np.float32))
    w1 = np.ascontiguousarray(np.asarray(w1, dtype=np.float32))
    w2 = np.ascontiguousarray(np.asarray(w2, dtype=np.float32))
    conv_w = np.asarray(conv_w, dtype=np.float32)

    convb = _build_conv_lhsT(conv_w)
    frames = f.reshape(FRAMES, C, HW)

    nc = _get_nc(FPC)
    in_maps = []
    for i in range(N_CORES):
        in_maps.append({
            "x": np.ascontiguousarray(frames[i * FPC:(i + 1) * FPC]),
            "w1": w1,
            "w2": w2,
            "convb": convb,
        })
    res = run_bass_kernel_spmd(nc, in_maps, core_ids=list(range(N_CORES)))
    out = np.concatenate([res.results[i]["out"] for i in range(N_CORES)], axis=0)
    return out.reshape(B, T, C, H, W)


if __name__ == "__main__":
    rng = np.random.default_rng(0)
    f = rng.standard_normal((B, T, C, H, W), dtype=np.float32)
    w1 = rng.standard_normal((C, 16), dtype=np.float32) / 16.0
    w2 = rng.standard_normal((16, C), dtype=np.float32) / 4.0
    conv_w = rng.standard_normal((1, 2, 7, 7), dtype=np.float32) * 0.1
    out = kernel(f, w1, w2, conv_w)
    print("kernel ran, out shape", out.shape, out.dtype)


# revision 17
# speedup vs baseline: 1.2785x; 1.2785x over previous
"""CBAM (channel + spatial attention) Trainium2 kernel, 8-core data parallel.

Problem: f [8, 8, 256, 56, 56] f32 -> out same shape.
  x = f.reshape(BT, C, H, W)
  ca = sigmoid(mlp(max_hw(x)) + mlp(mean_hw(x)));  xc = ca * x
  s  = conv7x7([mean_c(xc); max_c(xc)]);           out = sigmoid(s) * xc

Strategy (per NeuronCore, 8 frames each, no collectives):
  - channel-major layout [128 part(channels), free(hw=3136)], 2 channel tiles
  - phase1 (ACT): xb = bf16(x/HW) fused with accum_out -> channel avg
  - pool-max (DVE): log2 fold tree on xb (bf16 TT max at 2x rate)
  - MLP on PE (K-split matmuls), relu rescale trick absorbs the 1/HW scale
  - xc = ca*xb on DVE (bf16 tensor_scalar 4x)
  - spatial sum: PE matmul with ca as lhsT over xb (reduces channel/K dim)
  - spatial max: DVE channel fold + PE 128x112 transposes + DVE reduce from PSUM
  - conv 7x7 as 7 accumulating PE matmuls with host-built banded lhsT [124,56]
  - sa broadcast via PE ones(=HW)-matmul; final mul on DVE (bf16 TT 2x)
  - store via SWDGE dma cast bf16->f32
"""

import os
import sys
from contextlib import ExitStack

import numpy as np

if "/opt/trn_rl_repo" not in sys.path:
    sys.path.insert(0, "/opt/trn_rl_repo")

import concourse.bass as bass
import concourse.tile as tile
from concourse import bacc, mybir
from concourse.bass_utils import run_bass_kernel_spmd
from concourse.masks import make_identity

F32 = mybir.dt.float32
BF16 = mybir.dt.bfloat16

N_CORES = 8
B, T, C, H, W = 8, 8, 256, 56, 56
HW = H * W            # 3136
FRAMES = B * T        # 64
FPC = FRAMES // N_CORES  # frames per core = 8
PAD = 3
HP, WP = H + 2 * PAD, W + 2 * PAD  # 62, 62
CHK = 112             # hw chunk size for transposes (28 * 112 = 3136)
NCHUNK = HW // CHK    # 28
TGRP = 7              # transpose chunks per psum group
NGRP = NCHUNK // TGRP  # 4
SCHK = 448            # free-dim chunk for ssum / broadcast matmuls
NSCHK = HW // SCHK    # 7


def _build_conv_lhsT(conv_w: np.ndarray) -> np.ndarray:
    """Banded matrices for the 7x7 conv as 7 accumulating matmuls over y.

    B[dx] : [124, 56], rows = c*62 + y_in, cols = y_out.
    B[dx][c*62 + yi, yo] = w_eff[c, yi-yo, dx] for 0 <= yi-yo <= 6.
    The channel-mean 1/C is folded into the avg branch (c=0).
    """
    w_eff = conv_w[0].astype(np.float64).copy()  # [2, 7, 7]
    w_eff[0] /= C
    Bm = np.zeros((7, 2 * HP, H), dtype=np.float32)
    dyi = np.arange(7)
    for dx in range(7):
        for c in range(2):
            for yo in range(H):
                Bm[dx, c * HP + yo + dyi, yo] = w_eff[c, :, dx]
    return Bm


def build_nc(n_frames: int = FPC):
    nc = bacc.Bacc("TRN2", target_bir_lowering=False, debug=False,
                   num_devices=N_CORES)

    x_ext = nc.dram_tensor("x", [n_frames, C, HW], F32, kind="ExternalInput")
    w1_ext = nc.dram_tensor("w1", [C, 16], F32, kind="ExternalInput")
    w2_ext = nc.dram_tensor("w2", [16, C], F32, kind="ExternalInput")
    cb_ext = nc.dram_tensor("convb", [7, 2 * HP, H], F32, kind="ExternalInput")
    out_ext = nc.dram_tensor("out", [n_frames, C, HW], F32, kind="ExternalOutput")

    with tile.TileContext(nc) as tc, ExitStack() as ctx:
        consts = ctx.enter_context(tc.tile_pool(name="consts", bufs=1))
        xin = ctx.enter_context(tc.tile_pool(name="xin", bufs=2))
        xbp = ctx.enter_context(tc.tile_pool(name="xb", bufs=2))
        xcp = ctx.enter_context(tc.tile_pool(name="xc", bufs=2))
        obp = ctx.enter_context(tc.tile_pool(name="ob", bufs=2))
        fold = ctx.enter_context(tc.tile_pool(name="fold", bufs=1))
        m1p = ctx.enter_context(tc.tile_pool(name="m1", bufs=2))
        small = ctx.enter_context(tc.tile_pool(name="small", bufs=2))
        sap = ctx.enter_context(tc.tile_pool(name="sa", bufs=1))
        sabp = ctx.enter_context(tc.tile_pool(name="sab", bufs=2))
        # PSUM pools — total across all tags must stay within 8 banks.
        ptr = ctx.enter_context(tc.tile_pool(name="ptr", bufs=2, space="PSUM"))
        pz = ctx.enter_context(tc.tile_pool(name="pz", bufs=1, space="PSUM"))
        pss = ctx.enter_context(tc.tile_pool(name="pss", bufs=2, space="PSUM"))
        pbc = ctx.enter_context(tc.tile_pool(name="pbc", bufs=2, space="PSUM"))
        pmlp = ctx.enter_context(tc.tile_pool(name="pmlp", bufs=1, space="PSUM"))
        # bank budget: ptr 1x2 + pz 1 + pss 1x2 + pbc 1x2 + pmlp 1 = 8

        # ---- constants / weights (loaded once) ----
        w1_sb = consts.tile([128, 2, 16], F32)       # [k, ktile, m]
        for t in range(2):
            nc.sync.dma_start(w1_sb[:, t, :], w1_ext[t * 128:(t + 1) * 128, :])
        w2_sb = consts.tile([16, C], F32)
        nc.sync.dma_start(w2_sb[:], w2_ext[:, :])
        cb_sb = consts.tile([124, 7, H], BF16)       # [y_in(+c), dx, y_out]
        nc.gpsimd.dma_start(                          # SWDGE: casts f32->bf16
            cb_sb[:],
            cb_ext.rearrange("d p y -> p d y"),
        )
        ident_b = consts.tile([128, 128], BF16)
        make_identity(nc, ident_b[:])
        ident_f = consts.tile([128, 128], F32)
        make_identity(nc, ident_f[:])
        ones_hw = consts.tile([1, 128], BF16)        # value HW for sa broadcast
        nc.vector.memset(ones_hw[:], float(HW))

        for f in range(n_frames):
            # ---------- load frame ----------
            x_sb = xin.tile([128, 2, HW], F32, tag="x")
            for t in range(2):
                nc.sync.dma_start(
                    x_sb[:, t, :], x_ext[f, t * 128:(t + 1) * 128, :])

            # ---------- phase 1: cast+scale + channel-avg (ACT) ----------
            xb = xbp.tile([128, 2, HW], BF16, tag="xb")
            # pr: flat [128, 4]; col 2t = max of tile t, col 2t+1 = avg of tile t
            pr = small.tile([128, 4], F32, tag="pr")
            for t in range(2):
                nc.scalar.activation(
                    xb[:, t, :], x_sb[:, t, :],
                    mybir.ActivationFunctionType.Copy,
                    scale=1.0 / HW,
                    accum_out=pr[:, 2 * t + 1:2 * t + 2])

            # ---------- pool-max: fold tree on xb (DVE) ----------
            cur = xb
            width = HW
            while width > 98:
                half = width // 2
                nxt = fold.tile([128, 2, half], BF16, tag=f"fold{half}")
                nc.vector.tensor_tensor(
                    out=nxt[:, :, :],
                    in0=cur[:, :, 0:half],
                    in1=cur[:, :, half:2 * half],
                    op=mybir.AluOpType.max)
                cur = nxt
                width = half
            nc.vector.tensor_reduce(
                out=pr.rearrange("p (t two) -> p t two", two=2)[:, :, 0],
                in_=cur[:, :, :],
                axis=mybir.AxisListType.X,
                op=mybir.AluOpType.max)

            # ---------- MLP on PE ----------
            ph = pmlp.tile([16, 2], F32, tag="pmlp")
            for t in range(2):
                nc.tensor.matmul(ph[:], w1_sb[:, t, :],
                                 pr[:, 2 * t:2 * t + 2],
                                 start=(t == 0), stop=(t == 1))
            h = small.tile([16, 2], F32, tag="h")
            # col 0 (max branch) needs the 1/HW scale undone before relu
            nc.scalar.activation(h[:, 0:1], ph[:, 0:1],
                                 mybir.ActivationFunctionType.Relu,
                                 scale=float(HW))
            nc.scalar.activation(h[:, 1:2], ph[:, 1:2],
                                 mybir.ActivationFunctionType.Relu)
            hs = small.tile([16, 1], F32, tag="hs")
            nc.vector.tensor_tensor(out=hs[:], in0=h[:, 0:1], in1=h[:, 1:2],
                                    op=mybir.AluOpType.add)
            pca = pmlp.tile([128, 2], F32, tag="pmlp")
            for t in range(2):
                nc.tensor.matmul(pca[:, t:t + 1],
                                 w2_sb[:, t * 128:(t + 1) * 128], hs[:],
                                 start=True, stop=True)
            ca = small.tile([128, 2], F32, tag="ca")
            nc.scalar.activation(ca[:], pca[:],
                                 mybir.ActivationFunctionType.Sigmoid)
            ca_b = small.tile([128, 2], BF16, tag="ca_b")
            nc.vector.tensor_copy(ca_b[:], ca[:])

            # ---------- xc = ca * xb (DVE, bf16 4x) ----------
            xc = xcp.tile([128, 2, HW], BF16, tag="xc")
            for t in range(2):
                nc.vector.tensor_scalar(
                    out=xc[:, t, :], in0=xb[:, t, :],
                    scalar1=ca[:, t:t + 1], scalar2=None,
                    op0=mybir.AluOpType.mult)

            # ---------- spatial sum via PE (ca as lhsT over xb) ----------
            ssum_row = sap.tile([1, HW], BF16, tag="ssum_row")
            for j in range(NSCHK):
                psc = pss.tile([1, SCHK], F32, tag="pss")
                for t in range(2):
                    nc.tensor.matmul(
                        psc[:],
                        ca_b[:, t:t + 1],
                        xb[:, t, j * SCHK:(j + 1) * SCHK],
                        start=(t == 0), stop=(t == 1))
                nc.scalar.activation(ssum_row[:, j * SCHK:(j + 1) * SCHK],
                                     psc[:],
                                     mybir.ActivationFunctionType.Copy)

            # ---------- spatial max: channel fold + transposes ----------
            m1 = m1p.tile([128, HW], BF16, tag="m1")
            nc.vector.tensor_tensor(out=m1[:], in0=xc[:, 0, :],
                                    in1=xc[:, 1, :], op=mybir.AluOpType.max)
            sm_cols = small.tile([CHK, NCHUNK], F32, tag="sm_cols")
            for g in range(NGRP):
                pt = ptr.tile([CHK, TGRP, 128], BF16, tag="ptr")
                for j in range(TGRP):
                    c = g * TGRP + j
                    nc.tensor.transpose(
                        pt[:, j, :], m1[:, c * CHK:(c + 1) * CHK], ident_b[:])
                nc.vector.tensor_reduce(
                    out=sm_cols[:, g * TGRP:(g + 1) * TGRP],
                    in_=pt[:, :, :],
                    axis=mybir.AxisListType.X,
                    op=mybir.AluOpType.max)

            # sm_cols [112, 28] -> transpose -> [28, 112] -> sbuf (hw order)
            psm = pz.tile([NCHUNK, CHK], F32, tag="pz")
            nc.tensor.transpose(psm[:], sm_cols[:], ident_f[0:CHK, 0:CHK])
            sm_sb = small.tile([NCHUNK, CHK], BF16, tag="sm_sb")
            nc.scalar.activation(sm_sb[:], psm[:],
                                 mybir.ActivationFunctionType.Copy)

            # ---------- conv input assembly ----------
            s_pad = sap.tile([124, WP], BF16, tag="s_pad")
            nc.vector.memset(s_pad[:], 0.0)
            # avg rows (c=0): partitions 3..58; max rows (c=1): 65..120
            # element-count-matched SBUF->SBUF fold DMAs (hw-sequential streams)
            nc.sync.dma_start(s_pad[PAD:PAD + H, PAD:PAD + W], ssum_row[:])
            nc.sync.dma_start(s_pad[HP + PAD:HP + PAD + H, PAD:PAD + W],
                              sm_sb[:])

            # ---------- conv: 7 accumulating matmuls ----------
            pzt = pz.tile([H, W], F32, tag="pz")
            for dx in range(7):
                nc.tensor.matmul(pzt[:], cb_sb[:, dx, :],
                                 s_pad[:, dx:dx + W],
                                 start=(dx == 0), stop=(dx == 6))
            sa_yx = small.tile([H, W], BF16, tag="sa_yx")
            nc.scalar.activation(sa_yx[:], pzt[:],
                                 mybir.ActivationFunctionType.Sigmoid,
                                 scale=float(HW))

            # ---------- sa broadcast ----------
            sa_row = sap.tile([1, HW], BF16, tag="sa_row")
            nc.sync.dma_start(sa_row[:], sa_yx[:])
            sab = sabp.tile([128, HW], BF16, tag="sab")
            for j in range(NSCHK):
                pb = pbc.tile([128, SCHK], F32, tag="pbc")
                nc.tensor.matmul(pb[:], ones_hw[:],
                                 sa_row[:, j * SCHK:(j + 1) * SCHK],
                                 start=True, stop=True)
                nc.scalar.activation(sab[:, j * SCHK:(j + 1) * SCHK], pb[:],
                                     mybir.ActivationFunctionType.Copy)

            # ---------- final: out = xc * (HW * sa) ----------
            ob = obp.tile([128, 2, HW], BF16, tag="ob")
            for t in range(2):
                nc.vector.tensor_tensor(out=ob[:, t, :], in0=xc[:, t, :],
                                        in1=sab[:], op=mybir.AluOpType.mult)
                nc.gpsimd.dma_start(
                    out_ext[f, t * 128:(t + 1) * 128, :], ob[:, t, :])

    nc.finalize()  # bacc register allocation + DCE (bass2jax expects this)
    return nc


_NC_CACHE = {}


def _get_nc(n_frames: int):
    if n_frames not in _NC_CACHE:
        _NC_CACHE[n_frames] = build_nc(n_frames)
    return _NC_CACHE[n_frames]


def kernel(f: np.ndarray, w1: np.ndarray, w2: np.ndarray,
           conv_w: np.ndarray) -> np.ndarray:
    f = np.ascontiguousarray(np.asarray(f, dtype=np.float32))
    w1 = np.ascontiguousarray(np.asarray(w1, dtype=np.float32))
    w2 = np.ascontiguousarray(np.asarray(w2, dtype=np.float32))
    conv_w = np.asarray(conv_w, dtype=np.float32)

    convb = _build_conv_lhsT(conv_w)
    frames = f.reshape(FRAMES, C, HW)

    nc = _get_nc(FPC)
    in_maps = []
    for i in range(N_CORES):
        in_maps.append({
            "x": np.ascontiguousarray(frames[i * FPC:(i + 1) * FPC]),
            "w1": w1,
            "w2": w2,
            "convb": convb,
        })
    res = run_bass_kernel_spmd(nc, in_maps, core_ids=list(range(N_CORES)))
    out = np.concatenate([res.results[i]["out"] for i in range(N_CORES)], axis=0)
    return out.reshape(B, T, C, H, W)


if __name__ == "__main__":
    rng = np.random.default_rng(0)
    f = rng.standard_normal((B, T, C, H, W), dtype=np.float32)
    w1 = rng.standard_normal((C, 16), dtype=np.float32) / 16.0
    w2 = rng.standard_normal((16, C), dtype=np.float32) / 4.0
    conv_w = rng.standard_normal((1, 2, 7, 7), dtype=np.float32) * 0.1
    out = kernel(f, w1, w2, conv_w)
    print("kernel ran, out shape", out.shape, out.dtype)
